# revision 38
# baseline (speedup 1.0000x reference)
"""LM head log_softmax kernel for 8 Trainium2 NeuronCores.

Computes log_softmax(h @ W^T) for h [2,2048,1024] f32, W [50257,1024] f32.

Strategy (tensor parallel over vocab), v3:
  - W is sharded along vocab across 8 cores (6288 padded cols each,
    8*6288 = 50304 >= 50257).
  - Per 128-token m-tile, each core matmuls its vocab shard in fp8
    (DoubleRow, psum f32), the DVE drains psum -> SBUF as bf16 *logits*
    (tensor_scalar mul by 1/W_SCALE), and the scalar engine makes one
    full-width Exp pass over the staged logits whose accum_out gives the
    per-row sum of exps (the elementwise output goes to a throwaway
    scratch tile).  Row sums for a block of m-tiles are all-reduced
    across the 8 cores (tiny [128,<=4] f32 collective per block),
    lse = Ln(global sum) on the scalar engine, and pass 2 is a 4x-mode
    DVE tensor_scalar_sub: out = logit - lse, written bf16 and DMA'd
    straight out.  Host upcasts to f32.
  - psum is a ring of 4 x 2-bank tiles (1024 f32): drains are 1.19us
    and stay well ahead of the PE (a 2x 4-bank ring stalled the PE
    ~2.8us at every m-tile boundary waiting for a 2.75us drain).
  - Pass 2 / Ln for block b are emitted after pass 1 of block b+1 so the
    ~30us AllReduce latency hides behind a block of compute.  A dummy
    AllReduce at kernel start absorbs the collective bootstrap barrier.
    Trailing blocks shrink (4,...,4,2,1,1) so the unavoidable tail
    (last Exp -> AllReduce -> subtract -> DMA) covers 1 m-tile, not 4.
  - Vocab padding (zero W rows -> logit 0 -> exp 1) is corrected by a
    host-supplied additive adjustment to the local row sums (-n_pad on
    the last core), exact since exp(0) == 1 in every dtype.

Host side: transposes h and the W shard to K-major (fp8), launches the
SPMD kernel via run_bass_kernel_spmd on cores 0-7, concatenates the
per-core [4096, 6288] bf16 outputs along vocab, slices off the padding
and upcasts to f32.
"""

import os

import numpy as np
import ml_dtypes

import concourse.bass as bass
import concourse.bacc as bacc
import concourse.mybir as mybir
import concourse.tile as tile
from concourse.bass_utils import run_bass_kernel_spmd

N_CORES = 8
B, S, D = 2, 2048, 1024
T = B * S                      # 4096 tokens
V = 50257
VC = 6288                      # per-core padded vocab shard (8*6288 = 50304)
P = 128                        # SBUF partitions
K_TILES = D // P               # 8
K_PAIRS = K_TILES // 2         # 4 (DoubleRow consumes 2 k-tiles per matmul)
M_TILES = T // P               # 32
BLOCKS = [4, 4, 4, 4, 4, 4, 4, 3, 1]   # m-tiles per collective block
assert sum(BLOCKS) == M_TILES
BLK_MAX = max(BLOCKS)
# psum groups: 2 banks (1024 f32) per drain; 6*1024 + 144 tail = 6288
GROUPS = [(i * 1024, 1024) for i in range(6)] + [(6144, 144)]
CHUNK = 512                    # matmul moving-operand max for f32 psum

BF16 = mybir.dt.bfloat16
F32 = mybir.dt.float32
FP8 = mybir.dt.float8e4
NP_FP8 = mybir.dt.np(mybir.dt.float8e4)
W_SCALE = 32.0

# results of the last run_bass_kernel_spmd call (for test harness inspection)
LAST_RESULT = None


def build_nc():
    nc = bacc.Bacc(
        "TRN2",
        target_bir_lowering=False,
        debug=False,
        num_devices=N_CORES,
    )
    hT = nc.dram_tensor("hT", [D, T], FP8, kind="ExternalInput").ap()
    wT = nc.dram_tensor("wT", [D, VC], FP8, kind="ExternalInput").ap()
    out = nc.dram_tensor("out", [T, VC], BF16, kind="ExternalOutput").ap()

    # K-major views with the partition dim innermost of K: [128, K_TILES, *]
    hT_r = hT.rearrange("(k p) m -> p k m", p=P)
    wT_r = wT.rearrange("(k p) n -> p k n", p=P)

    groups_ar = [list(range(N_CORES))]

    with tile.TileContext(nc) as tc:
        with (
            tc.tile_pool(name="singles", bufs=1) as singles,
            tc.tile_pool(name="hts", bufs=6) as hts_pool,
            tc.tile_pool(name="psum", bufs=4, space="PSUM") as psum_pool,
            tc.tile_pool(name="logits", bufs=10) as logits_pool,
            tc.tile_pool(name="stats", bufs=4) as stats_pool,
            tc.tile_pool(name="cc", bufs=4, space="DRAM") as cc_pool,
        ):
            # hoist the first m-tiles' activations ahead of the W preload:
            # DMA descriptors drain in issue order per queue, so anything
            # emitted after the 6.3MB W preload waits ~18us behind it.
            # Afterwards each m-tile's ht is issued 4 m-tiles early, keeping
            # ht descriptors ahead of the pass-2 output DMAs in the queue.
            HT_AHEAD = 4
            ht_pre = {}

            def issue_ht(m):
                ht = hts_pool.tile([P, K_TILES, P], FP8, name=f"ht{m}", tag="ht")
                nc.sync.dma_start(out=ht, in_=hT_r[:, :, m * P : (m + 1) * P])
                ht_pre[m] = ht

            for m in range(HT_AHEAD):
                issue_ht(m)
            # W preload in k-order halves so the first accumulation pairs'
            # weights land early; spread across the sync/scalar/vector DMA
            # trigger queues (all idle at t=0) for parallel drain
            wt_sb = singles.tile([P, K_TILES, VC], FP8)
            H = VC // 2
            w_engs = [nc.sync, nc.scalar]
            qi = 0
            for k in range(K_TILES):
                for h0 in (0, H):
                    w_engs[qi % len(w_engs)].dma_start(
                        out=wt_sb[:, k, h0 : h0 + H], in_=wT_r[:, k, h0 : h0 + H]
                    )
                    qi += 1
            # throwaway destination for the Exp pass (only accum_out is used)
            exp_scratch = singles.tile([P, VC], BF16)
            # total vocab padding (8*VC - V zero W rows, exp(0)=1 each) is a
            # compile-time constant, applied as a pre-Ln bias
            neg_pad = singles.tile([P, 1], F32)
            nc.vector.memset(neg_pad, -float(N_CORES * VC - V))

            # dummy collective: absorbs the mesh bootstrap barrier while
            # the first blocks of real compute run (payload bytes are junk)
            dummy_in = cc_pool.tile([P, 1], F32, tag="dummy_in", bufs=1)
            dummy_out = cc_pool.tile([N_CORES * P, 1], F32, tag="dummy_out", bufs=1)
            nc.gpsimd.dma_start(
                out=dummy_in.bitcast(FP8), in_=wT_r[0:P, 0, 0:4]
            )
            nc.gpsimd.collective_compute(
                "AllGather",
                mybir.AluOpType.bypass,
                replica_groups=groups_ar,
                ins=[dummy_in[:, :].opt()],
                outs=[dummy_out[:, :].opt()],
            )

            def emit_pass2(lgs_p, gsums_p, m0_p):
                # lse = Ln(global sums) — tiny [128, <=4] ACT op
                bs = len(lgs_p)
                lse = stats_pool.tile([P, BLK_MAX], F32, name="lse")
                nc.scalar.activation(
                    out=lse[:, :bs],
                    in_=gsums_p,
                    func=mybir.ActivationFunctionType.Ln,
                    bias=neg_pad[:, 0:1],
                )
                for mb in range(bs):
                    m = m0_p + mb
                    # in-place: out = logit - lse overwrites the staged logits,
                    # the DMA streams them out, then the slot recycles
                    nc.vector.tensor_scalar_sub(
                        out=lgs_p[mb], in0=lgs_p[mb], scalar1=lse[:, mb : mb + 1]
                    )
                    nc.sync.dma_start(
                        out=out[m * P : (m + 1) * P, :], in_=lgs_p[mb]
                    )

            pendings = []
            m0 = 0
            for blk, bs in enumerate(BLOCKS):
                raw_sums = stats_pool.tile([P, BLK_MAX], F32, name="raw_sums")
                lgs = []
                for mb in range(bs):
                    m = m0 + mb
                    if m + HT_AHEAD < M_TILES:
                        issue_ht(m + HT_AHEAD)
                    ht = ht_pre.pop(m)
                    # emit the epilogue of block b-2 after this block's first
                    # m-tile: the DVE/ACT queues reach it ~1.25 blocks (~70us)
                    # after block b-2's AllReduce was triggered, covering even
                    # skew-inflated (~50us) collective latencies without
                    # head-of-line blocking
                    if mb == min(1, bs - 1) and len(pendings) >= 2:
                        emit_pass2(*pendings.pop(0))
                    lg = logits_pool.tile([P, VC], BF16, tag="lg", name="lg")
                    for g0, gs in GROUPS:
                        ps = psum_pool.tile([P, 1024], F32, tag="ps", name="ps")
                        for kp in range(K_PAIRS):
                            for c0 in range(g0, g0 + gs, CHUNK):
                                cs = min(CHUNK, g0 + gs - c0)
                                nc.tensor.matmul(
                                    out=ps[:, c0 - g0 : c0 - g0 + cs],
                                    lhsT=ht[:, 2 * kp : 2 * kp + 2, :],
                                    rhs=wt_sb[:, 2 * kp : 2 * kp + 2, c0 : c0 + cs],
                                    start=(kp == 0),
                                    stop=(kp == K_PAIRS - 1),
                                    perf_mode=mybir.MatmulPerfMode.DoubleRow,
                                )
                        # drain psum -> bf16 logits (descale by W_SCALE)
                        nc.vector.tensor_scalar_mul(
                            out=lg[:, g0 : g0 + gs],
                            in0=ps[:, :gs],
                            scalar1=1.0 / W_SCALE,
                        )
                    # one full-width Exp; only the per-row accumulation is kept
                    nc.scalar.activation(
                        out=exp_scratch[:, :],
                        in_=lg[:, :],
                        func=mybir.ActivationFunctionType.Exp,
                        accum_out=raw_sums[:, mb : mb + 1],
                    )
                    lgs.append(lg)

                # AllGather (lower latency floor than AllReduce) + local
                # 8-way reduce on the DVE.  Collectives need contiguous DRAM
                # APs — size tiles exactly.
                cc_in = cc_pool.tile([P, bs], F32, tag="cc_in")
                cc_out = cc_pool.tile([N_CORES * P, bs], F32, tag="cc_out")
                nc.gpsimd.dma_start(out=cc_in[:, :], in_=raw_sums[:, :bs])
                nc.gpsimd.collective_compute(
                    "AllGather",
                    mybir.AluOpType.bypass,
                    replica_groups=groups_ar,
                    ins=[cc_in[:, :].opt()],
                    outs=[cc_out[:, :].opt()],
                )
                gs8 = stats_pool.tile([P, BLK_MAX, N_CORES], F32, name="gs8")
                nc.gpsimd.dma_start(
                    out=gs8[:, :bs, :],
                    in_=cc_out.rearrange("(r p) b -> p b r", p=P),
                )
                gsums = stats_pool.tile([P, BLK_MAX], F32, name="gsums")
                nc.vector.tensor_reduce(
                    out=gsums[:, :bs],
                    in_=gs8[:, :bs, :],
                    axis=mybir.AxisListType.X,
                    op=mybir.AluOpType.add,
                )

                # (epilogues are emitted two blocks later, inside pass 1 above)
                pendings.append((lgs, gsums[:, :bs], m0))
                m0 += bs
            for p in pendings:
                emit_pass2(*p)
    nc.compile()
    return nc


def _prep_inputs(hidden_states, W):
    """Host-side shard + transpose + cast. Returns per-core input maps."""
    hflat = np.asarray(hidden_states, dtype=np.float32).reshape(T, D)
    hT = np.ascontiguousarray(hflat.T).astype(NP_FP8)

    W = np.asarray(W, dtype=np.float32)
    in_maps = []
    for c in range(N_CORES):
        lo, hi = c * VC, (c + 1) * VC
        shard = W[lo : min(hi, V)]
        wT_c = np.zeros((D, VC), dtype=NP_FP8)
        wT_c[:, : shard.shape[0]] = (shard.T * W_SCALE).astype(NP_FP8)
        in_maps.append({"hT": hT, "wT": wT_c})
    return in_maps


def kernel(hidden_states, W):
    global LAST_RESULT
    in_maps = _prep_inputs(hidden_states, W)
    nc = build_nc()
    trace = os.environ.get("LMHEAD_TRACE", "0") == "1"
    res = run_bass_kernel_spmd(
        nc, in_maps, list(range(N_CORES)), trace=trace
    )
    LAST_RESULT = res
    parts = [np.asarray(res.results[c]["out"]) for c in range(N_CORES)]
    full = np.concatenate(parts, axis=1)[:, :V].astype(np.float32)
    return np.ascontiguousarray(full.reshape(B, S, V))


# revision 44
# speedup vs baseline: 1.1767x; 1.1767x over previous
"""LM head log_softmax kernel for 8 Trainium2 NeuronCores.

Computes log_softmax(h @ W^T) for h [2,2048,1024] f32, W [50257,1024] f32.

Strategy (tensor parallel over vocab), v3:
  - W is sharded along vocab across 8 cores (6288 padded cols each,
    8*6288 = 50304 >= 50257).
  - Per 128-token m-tile, each core matmuls its vocab shard in fp8
    (DoubleRow, psum f32), the DVE drains psum -> SBUF as bf16 *logits*
    (tensor_scalar mul by 1/W_SCALE), and the scalar engine makes one
    full-width Exp pass over the staged logits whose accum_out gives the
    per-row sum of exps (the elementwise output goes to a throwaway
    scratch tile).  Row sums for a block of m-tiles are all-reduced
    across the 8 cores (tiny [128,<=4] f32 collective per block),
    lse = Ln(global sum) on the scalar engine, and pass 2 is a 4x-mode
    DVE tensor_scalar_sub: out = logit - lse, written bf16 and DMA'd
    straight out.  Host upcasts to f32.
  - psum is a ring of 4 x 2-bank tiles (1024 f32): drains are 1.19us
    and stay well ahead of the PE (a 2x 4-bank ring stalled the PE
    ~2.8us at every m-tile boundary waiting for a 2.75us drain).
  - Pass 2 / Ln for block b are emitted after pass 1 of block b+1 so the
    ~30us AllReduce latency hides behind a block of compute.  A dummy
    AllReduce at kernel start absorbs the collective bootstrap barrier.
    Trailing blocks shrink (4,...,4,2,1,1) so the unavoidable tail
    (last Exp -> AllReduce -> subtract -> DMA) covers 1 m-tile, not 4.
  - Vocab padding (zero W rows -> logit 0 -> exp 1) is corrected by a
    host-supplied additive adjustment to the local row sums (-n_pad on
    the last core), exact since exp(0) == 1 in every dtype.

Host side: transposes h and the W shard to K-major (fp8), launches the
SPMD kernel via run_bass_kernel_spmd on cores 0-7, concatenates the
per-core [4096, 6288] bf16 outputs along vocab, slices off the padding
and upcasts to f32.
"""

import os

import numpy as np
import ml_dtypes

import concourse.bass as bass
import concourse.bacc as bacc
import concourse.mybir as mybir
import concourse.tile as tile
from concourse.bass_utils import run_bass_kernel_spmd

N_CORES = 8
B, S, D = 2, 2048, 1024
T = B * S                      # 4096 tokens
V = 50257
VC = 6288                      # per-core padded vocab shard (8*6288 = 50304)
P = 128                        # SBUF partitions
K_TILES = D // P               # 8
K_PAIRS = K_TILES // 2         # 4 (DoubleRow consumes 2 k-tiles per matmul)
M_TILES = T // P               # 32
BLOCKS = [4, 4, 4, 4, 4, 4, 4, 3, 1]   # m-tiles per collective block
assert sum(BLOCKS) == M_TILES
BLK_MAX = max(BLOCKS)
# psum groups: 2 banks (1024 f32) per drain; 6*1024 + 144 tail = 6288
GROUPS = [(i * 1024, 1024) for i in range(6)] + [(6144, 144)]
CHUNK = 512                    # matmul moving-operand max for f32 psum

BF16 = mybir.dt.bfloat16
F32 = mybir.dt.float32
FP8 = mybir.dt.float8e4
NP_FP8 = mybir.dt.np(mybir.dt.float8e4)
W_SCALE = 32.0

# results of the last run_bass_kernel_spmd call (for test harness inspection)
LAST_RESULT = None


def build_nc():
    nc = bacc.Bacc(
        "TRN2",
        target_bir_lowering=False,
        debug=False,
        num_devices=N_CORES,
    )
    hT = nc.dram_tensor("hT", [D, T], FP8, kind="ExternalInput").ap()
    wT = nc.dram_tensor("wT", [D, VC], FP8, kind="ExternalInput").ap()
    out = nc.dram_tensor("out", [T, VC], BF16, kind="ExternalOutput").ap()

    # K-major views with the partition dim innermost of K: [128, K_TILES, *]
    hT_r = hT.rearrange("(k p) m -> p k m", p=P)
    wT_r = wT.rearrange("(k p) n -> p k n", p=P)

    groups_ar = [list(range(N_CORES))]

    with tile.TileContext(nc) as tc:
        with (
            tc.tile_pool(name="singles", bufs=1) as singles,
            tc.tile_pool(name="hts", bufs=6) as hts_pool,
            tc.tile_pool(name="psum", bufs=4, space="PSUM") as psum_pool,
            tc.tile_pool(name="logits", bufs=11) as logits_pool,
            tc.tile_pool(name="stats", bufs=4) as stats_pool,
            tc.tile_pool(name="cc", bufs=4, space="DRAM") as cc_pool,
        ):
            # hoist the first m-tiles' activations ahead of the W preload:
            # DMA descriptors drain in issue order per queue, so anything
            # emitted after the 6.3MB W preload waits ~18us behind it.
            # Afterwards each m-tile's ht is issued 4 m-tiles early, keeping
            # ht descriptors ahead of the pass-2 output DMAs in the queue.
            HT_AHEAD = 4
            ht_pre = {}

            def issue_ht(m):
                ht = hts_pool.tile([P, K_TILES, P], FP8, name=f"ht{m}", tag="ht")
                nc.sync.dma_start(out=ht, in_=hT_r[:, :, m * P : (m + 1) * P])
                ht_pre[m] = ht

            for m in range(HT_AHEAD):
                issue_ht(m)
            # W preload in k-order halves so the first accumulation pairs'
            # weights land early; spread across the sync/scalar/vector DMA
            # trigger queues (all idle at t=0) for parallel drain
            wt_sb = singles.tile([P, K_TILES, VC], FP8)
            H = VC // 2
            for k in range(K_TILES):
                for h0 in (0, H):
                    nc.sync.dma_start(
                        out=wt_sb[:, k, h0 : h0 + H], in_=wT_r[:, k, h0 : h0 + H]
                    )
            # throwaway destination for the Exp passes (only accum_out is
            # used); half-width, each m-tile runs two half-Exps whose two
            # partial sums ride the AllGather side by side
            VH = VC // 2
            exp_scratch = singles.tile([P, VH], BF16)
            # total vocab padding (8*VC - V zero W rows, exp(0)=1 each) is a
            # compile-time constant, applied as a pre-Ln bias
            neg_pad = singles.tile([P, 1], F32)
            nc.vector.memset(neg_pad, -float(N_CORES * VC - V))

            # dummy collective: absorbs the mesh bootstrap barrier while
            # the first blocks of real compute run (payload bytes are junk)
            dummy_in = cc_pool.tile([P, 1], F32, tag="dummy_in", bufs=1)
            dummy_out = cc_pool.tile([N_CORES * P, 1], F32, tag="dummy_out", bufs=1)
            nc.gpsimd.dma_start(
                out=dummy_in.bitcast(FP8), in_=wT_r[0:P, 0, 0:4]
            )
            nc.gpsimd.collective_compute(
                "AllGather",
                mybir.AluOpType.bypass,
                replica_groups=groups_ar,
                ins=[dummy_in[:, :].opt()],
                outs=[dummy_out[:, :].opt()],
            )

            def emit_pass2(lgs_p, gsums_p, m0_p):
                # lse = Ln(global sums) — tiny [128, <=4] ACT op
                bs = len(lgs_p)
                lse = stats_pool.tile([P, BLK_MAX], F32, name="lse")
                nc.scalar.activation(
                    out=lse[:, :bs],
                    in_=gsums_p,
                    func=mybir.ActivationFunctionType.Ln,
                    bias=neg_pad[:, 0:1],
                )
                for mb in range(bs):
                    m = m0_p + mb
                    # in-place: out = logit - lse overwrites the staged logits,
                    # the DMA streams them out, then the slot recycles
                    nc.vector.tensor_scalar_sub(
                        out=lgs_p[mb], in0=lgs_p[mb], scalar1=lse[:, mb : mb + 1]
                    )
                    nc.sync.dma_start(
                        out=out[m * P : (m + 1) * P, :], in_=lgs_p[mb]
                    )

            pendings = []
            m0 = 0
            for blk, bs in enumerate(BLOCKS):
                raw_sums = stats_pool.tile([P, 2 * BLK_MAX], F32, name="raw_sums")
                lgs = []
                for mb in range(bs):
                    m = m0 + mb
                    if m + HT_AHEAD < M_TILES:
                        issue_ht(m + HT_AHEAD)
                    ht = ht_pre.pop(m)
                    # emit the epilogue of block b-2 after this block's first
                    # m-tile: the DVE/ACT queues reach it ~1.25 blocks (~70us)
                    # after block b-2's AllReduce was triggered, covering even
                    # skew-inflated (~50us) collective latencies without
                    # head-of-line blocking
                    if mb == min(1, bs - 1) and len(pendings) >= 2:
                        emit_pass2(*pendings.pop(0))
                    lg = logits_pool.tile([P, VC], BF16, tag="lg", name="lg")
                    for g0, gs in GROUPS:
                        ps = psum_pool.tile([P, 1024], F32, tag="ps", name="ps")
                        for kp in range(K_PAIRS):
                            for c0 in range(g0, g0 + gs, CHUNK):
                                cs = min(CHUNK, g0 + gs - c0)
                                nc.tensor.matmul(
                                    out=ps[:, c0 - g0 : c0 - g0 + cs],
                                    lhsT=ht[:, 2 * kp : 2 * kp + 2, :],
                                    rhs=wt_sb[:, 2 * kp : 2 * kp + 2, c0 : c0 + cs],
                                    start=(kp == 0),
                                    stop=(kp == K_PAIRS - 1),
                                    perf_mode=mybir.MatmulPerfMode.DoubleRow,
                                )
                        # drain psum -> bf16 logits (descale by W_SCALE)
                        nc.vector.tensor_scalar_mul(
                            out=lg[:, g0 : g0 + gs],
                            in0=ps[:, :gs],
                            scalar1=1.0 / W_SCALE,
                        )
                    # two half-width Exps; only the per-row accumulations are
                    # kept (summed after the AllGather with the rank axis)
                    for h in range(2):
                        nc.scalar.activation(
                            out=exp_scratch[:, :],
                            in_=lg[:, h * VH : (h + 1) * VH],
                            func=mybir.ActivationFunctionType.Exp,
                            accum_out=raw_sums[:, 2 * mb + h : 2 * mb + h + 1],
                        )
                    lgs.append(lg)

                # AllGather (lower latency floor than AllReduce) of the 2
                # half-sums per m-tile + one local 16-way reduce on the DVE.
                # Collectives need contiguous DRAM APs — size tiles exactly.
                cc_in = cc_pool.tile([P, 2 * bs], F32, tag="cc_in")
                cc_out = cc_pool.tile([N_CORES * P, 2 * bs], F32, tag="cc_out")
                nc.gpsimd.dma_start(out=cc_in[:, :], in_=raw_sums[:, : 2 * bs])
                nc.gpsimd.collective_compute(
                    "AllGather",
                    mybir.AluOpType.bypass,
                    replica_groups=groups_ar,
                    ins=[cc_in[:, :].opt()],
                    outs=[cc_out[:, :].opt()],
                )
                gs8 = stats_pool.tile([P, BLK_MAX, N_CORES, 2], F32, name="gs8")
                nc.gpsimd.dma_start(
                    out=gs8[:, :bs, :, :],
                    in_=cc_out.rearrange("(r p) (b t) -> p b r t", p=P, t=2),
                )
                gsums = stats_pool.tile([P, BLK_MAX], F32, name="gsums")
                nc.vector.tensor_reduce(
                    out=gsums[:, :bs],
                    in_=gs8[:, :bs, :, :].rearrange("p b r t -> p b (r t)"),
                    axis=mybir.AxisListType.X,
                    op=mybir.AluOpType.add,
                )

                # (epilogues are emitted two blocks later, inside pass 1 above)
                pendings.append((lgs, gsums[:, :bs], m0))
                m0 += bs
            for p in pendings:
                emit_pass2(*p)
    nc.compile()
    return nc


def _prep_inputs(hidden_states, W):
    """Host-side shard + transpose + cast. Returns per-core input maps."""
    hflat = np.asarray(hidden_states, dtype=np.float32).reshape(T, D)
    hT = np.ascontiguousarray(hflat.T).astype(NP_FP8)

    W = np.asarray(W, dtype=np.float32)
    in_maps = []
    for c in range(N_CORES):
        lo, hi = c * VC, (c + 1) * VC
        shard = W[lo : min(hi, V)]
        wT_c = np.zeros((D, VC), dtype=NP_FP8)
        wT_c[:, : shard.shape[0]] = (shard.T * W_SCALE).astype(NP_FP8)
        in_maps.append({"hT": hT, "wT": wT_c})
    return in_maps


def kernel(hidden_states, W):
    global LAST_RESULT
    in_maps = _prep_inputs(hidden_states, W)
    nc = build_nc()
    trace = os.environ.get("LMHEAD_TRACE", "0") == "1"
    res = run_bass_kernel_spmd(
        nc, in_maps, list(range(N_CORES)), trace=trace
    )
    LAST_RESULT = res
    parts = [np.asarray(res.results[c]["out"]) for c in range(N_CORES)]
    full = np.concatenate(parts, axis=1)[:, :V].astype(np.float32)
    return np.ascontiguousarray(full.reshape(B, S, V))
